# revision 14
# baseline (speedup 1.0000x reference)
"""Trainium2 Bass kernel for a 2-layer ChebConv (K=2) GNN forward pass.

Math (reference):
    deg = bincount(row); dinv = where(deg>0, rsqrt(max(deg,1)), 0)
    z(M) = A_hat @ M with A_hat[r,c] = sum over edges (r,c) of -dinv[r]*dinv[c]
    h   = relu(x @ W1[0] + z(x @ W1[1]) + b1)
    out = log_softmax(h @ W2[0] + z(h) + ... @ W2[1] + b2)

Factorization used on device: z(M) = -dinv ⊙ (A @ (dinv ⊙ M)) so the edge
pass is a pure gather + segmented-sum (no per-edge weights).

Distribution: nodes row-sharded over 8 NeuronCores. Per layer each core
computes its u = dinv ⊙ (...) slice, an 8-rank AllGather builds the full
[N,16] gather table in HBM, then the core processes its own edges:
dma_gather (256B elements = 4 nodes' rows; edges bucketed by col&3 so the
wanted 16-float sub-row sits at a fixed offset per bucket) followed by an
affine DVE segmented reduction over a degree-class slot layout. Per-bucket
class orders are resolved by a second small gather (combine) per node.
The slot/class grid is shared across all 8 cores (SPMD: one program).
"""
import numpy as np

# problem constants (hardcoded per harness contract)
N = 100000
E = 1600000
FIN = 128
HID = 16
NC_N = 8
NPC = N // NC_N          # 12500 nodes per core
CHUNK = 1024             # idxs per dma_gather instruction (HW ring limit)
SEGROWS = 64             # gather-buffer window rows (1 row = 128 slots)
CHROWS = CHUNK // 128    # 8 rows per chunk
NQ = 4                   # SWDGE queues

_CACHE = {}


def _ceil(a, b):
    return (a + b - 1) // b


def _host_prep(edge_index):
    """Build per-core idx streams + the shared static class-grid structure."""
    row = np.asarray(edge_index[0], dtype=np.int64)
    col = np.asarray(edge_index[1], dtype=np.int64)
    deg = np.bincount(row, minlength=N).astype(np.float32)

    j_e = row // NPC                 # owning core of each edge
    b_e = (col & 3).astype(np.int64)  # bucket
    lr = row - j_e * NPC             # local dest id

    # per (core, bucket, local-node) degree
    key = (j_e * 4 + b_e) * NPC + lr
    d = np.bincount(key, minlength=NC_N * 4 * NPC).reshape(NC_N, 4, NPC)
    dmax = int(d.max())

    # class histogram per (core,bucket): n nodes with degree dd
    hist = np.zeros((NC_N, 4, dmax + 1), dtype=np.int64)
    for j in range(NC_N):
        for b in range(4):
            hist[j, b] = np.bincount(d[j, b], minlength=dmax + 1)

    # shared grid: blocks per class per bucket = max over cores
    B = np.zeros((4, dmax + 1), dtype=np.int64)
    for b in range(4):
        for dd in range(1, dmax + 1):
            B[b, dd] = _ceil(int(hist[:, b, dd].max()), 128)

    # pack blocks (in dd order) into SEGROWS-row segments; blocks never cross
    # a segment boundary. Build runs: (abs_row0, dd, nblk, zblk0).
    runs = [[] for _ in range(4)]     # per bucket
    totrows = [0] * 4
    nblk_tot = [0] * 4
    blockinfo = [{} for _ in range(4)]  # bucket -> dd -> (zblk0, [row0 of each blk])
    for b in range(4):
        cur = 0
        zblk = 0
        for dd in range(1, dmax + 1):
            nb = int(B[b, dd])
            if nb == 0:
                continue
            rows0 = []
            t = 0
            while t < nb:
                # how many blocks of dd rows fit in the current segment?
                room = SEGROWS - (cur % SEGROWS)
                fit = room // dd
                if fit == 0:
                    cur += room      # pad to segment boundary
                    continue
                take = min(fit, nb - t)
                runs[b].append((cur, dd, take, zblk + t))
                for k in range(take):
                    rows0.append(cur + k * dd)
                cur += take * dd
                t += take
            blockinfo[b][dd] = (zblk, rows0)
            zblk += nb
        cur = _ceil(cur, SEGROWS) * SEGROWS
        totrows[b] = cur
        nblk_tot[b] = zblk

    # per-core data: main gather idx streams + combine idx streams
    ZPAD_IDX = N // 4                 # table zero row index (25000)
    CPADCH = _ceil(NPC, CHUNK)        # 13 combine chunks per bucket
    CLEN = CPADCH * CHUNK
    gl = sum(tr * 128 for tr in totrows)
    gidx_all = np.empty((NC_N, 128, gl // 16), dtype=np.int16)
    cidx_all = np.empty((NC_N, 128, 4 * CLEN // 16), dtype=np.int16)

    # edge order: by (core, bucket, node) stable
    order = np.lexsort((lr, b_e, j_e))
    ro, bo, lo, co = j_e[order], b_e[order], lr[order], col[order]

    for j in range(NC_N):
        gparts = []
        cparts = []
        for b in range(4):
            slots = np.full(totrows[b] * 128, ZPAD_IDX, dtype=np.int64)
            sel = (ro == j) & (bo == b)
            nod = lo[sel]
            cv = co[sel] >> 2
            # rank of each edge within its node (nod sorted ascending)
            startd = np.searchsorted(nod, np.arange(NPC))
            cnt = d[j, b]
            jj = np.arange(nod.size) - np.repeat(startd, cnt)
            # node -> (block, p) assignment per class
            pi = np.full(NPC + 1, nblk_tot[b] * 128, dtype=np.int64)  # default: zero row
            rowstart_of_node = np.zeros(NPC, dtype=np.int64)
            for dd in range(1, dmax + 1):
                nodes_dd = np.where(cnt == dd)[0]
                if nodes_dd.size == 0:
                    continue
                zblk0, rows0 = blockinfo[b][dd]
                blk_i = np.arange(nodes_dd.size) // 128
                p_i = np.arange(nodes_dd.size) % 128
                pi[nodes_dd] = (zblk0 + blk_i) * 128 + p_i
                rowstart_of_node[nodes_dd] = np.asarray(rows0)[blk_i]
            p_of_node = np.zeros(NPC, dtype=np.int64)
            nz = cnt > 0
            p_of_node[nz] = pi[:NPC][nz] % 128
            pos = (rowstart_of_node[nod] + jj) * 128 + p_of_node[nod]
            slots[pos] = cv
            # wrap to [16, L/16] stream layout then replicate x8
            s16 = slots.reshape(-1, 16).T.astype(np.int16)
            gparts.append(np.tile(s16, (8, 1)))
            # combine stream: natural node order + pad
            cstream = np.full(CLEN, nblk_tot[b] * 128, dtype=np.int64)
            cstream[:NPC] = pi[:NPC]
            c16 = cstream.reshape(-1, 16).T.astype(np.int16)
            cparts.append(np.tile(c16, (8, 1)))
        gidx_all[j] = np.concatenate(gparts, axis=1)
        cidx_all[j] = np.concatenate(cparts, axis=1)

    struct = {
        "runs": runs,
        "totrows": totrows,
        "nblk": nblk_tot,
        "cpadch": CPADCH,
        "gl": gl,
    }
    return deg, gidx_all, cidx_all, struct


def _build(struct):
    import concourse.bacc as bacc
    import concourse.bass as bass
    import concourse.mybir as mybir
    import concourse.tile as tile
    from concourse.masks import make_identity

    runs = struct["runs"]
    totrows = struct["totrows"]
    nblk = struct["nblk"]
    CPADCH = struct["cpadch"]
    GL = struct["gl"]
    CLEN = CPADCH * CHUNK
    NPAD = _ceil(NPC, 128) * 128          # 12544
    NBCH = NPAD // 128                    # 98 node chunks
    ZROWS = [(nblk[b] + 1) * 128 for b in range(4)]
    NTABROW = N + 64                      # node rows in table (zero tail)

    f32 = mybir.dt.float32
    i16 = mybir.dt.int16
    AX = mybir.AxisListType
    OP = mybir.AluOpType
    AF = mybir.ActivationFunctionType

    nc = bacc.Bacc("TRN2", target_bir_lowering=False, debug=False,
                   num_devices=NC_N, num_swdge_queues=NQ)

    xt = nc.dram_tensor("xt", [128, NPAD], f32, kind="ExternalInput")
    degt = nc.dram_tensor("degt", [128, NBCH], f32, kind="ExternalInput")
    w1 = nc.dram_tensor("w1", [128, 32], f32, kind="ExternalInput")
    b1t = nc.dram_tensor("b1t", [128, HID], f32, kind="ExternalInput")
    w2 = nc.dram_tensor("w2", [48, FIN], f32, kind="ExternalInput")
    b2t = nc.dram_tensor("b2t", [128, FIN], f32, kind="ExternalInput")
    gidx = nc.dram_tensor("gidx", [128, GL // 16], i16, kind="ExternalInput")
    cidx = nc.dram_tensor("cidx", [128, 4 * CLEN // 16], i16, kind="ExternalInput")
    outp = nc.dram_tensor("outp", [NPAD, FIN], f32, kind="ExternalOutput")

    uin = nc.dram_tensor("uin", [NPAD, HID], f32, kind="Internal")
    utab = nc.dram_tensor("utab", [NTABROW, HID], f32, kind="Internal",
                          addr_space="Shared")
    zst = [nc.dram_tensor(f"zst{b}", [ZROWS[b], 64], f32, kind="Internal")
           for b in range(4)]

    utab_v = utab.rearrange("(r k) f -> r (k f)", k=4)   # [25016, 64]
    rg = [list(range(NC_N))]

    qctr = [0]

    def nextq():
        q = qctr[0] % NQ
        qctr[0] += 1
        return q

    with tile.TileContext(nc) as tc:
        with (
            tc.tile_pool(name="const", bufs=1) as cpool,
            tc.tile_pool(name="big", bufs=1) as bigpool,
            tc.tile_pool(name="nodes", bufs=1) as npool,
            tc.tile_pool(name="gseg", bufs=2) as gpool,
            tc.tile_pool(name="idx", bufs=6) as ipool,
            tc.tile_pool(name="gc", bufs=2) as gcpool,
            tc.tile_pool(name="sm", bufs=3) as smpool,
            tc.tile_pool(name="ps", bufs=2, space="PSUM") as pspool,
            tc.tile_pool(name="ps2", bufs=2, space="PSUM") as ps2pool,
        ):
            # ---- constants / inputs ----
            xtt = bigpool.tile([128, NPAD], f32, tag="big")
            nc.sync.dma_start(xtt[:], xt[:])
            w1t = cpool.tile([128, 32], f32)
            nc.sync.dma_start(w1t[:], w1[:])
            w2t = cpool.tile([48, FIN], f32)
            nc.sync.dma_start(w2t[:], w2[:])
            b1tt = cpool.tile([128, HID], f32)
            nc.sync.dma_start(b1tt[:], b1t[:])
            b2tt = cpool.tile([128, FIN], f32)
            nc.sync.dma_start(b2tt[:], b2t[:])
            degtt = cpool.tile([128, NBCH], f32)
            nc.sync.dma_start(degtt[:], degt[:])
            ident = cpool.tile([128, 128], f32)
            make_identity(nc, ident[:])

            # dinv = (deg>0) / sqrt(max(deg,1)); ndinv = -dinv
            mask = cpool.tile([128, NBCH], f32)
            nc.vector.tensor_scalar(out=mask[:], in0=degtt[:], scalar1=0.5,
                                    scalar2=None, op0=OP.is_ge)
            dmx = cpool.tile([128, NBCH], f32)
            nc.vector.tensor_scalar(out=dmx[:], in0=degtt[:], scalar1=1.0,
                                    scalar2=None, op0=OP.max)
            sq = cpool.tile([128, NBCH], f32)
            nc.scalar.activation(sq[:], dmx[:], AF.Sqrt)
            rs = cpool.tile([128, NBCH], f32)
            nc.vector.reciprocal(rs[:], sq[:])
            dinv = cpool.tile([128, NBCH], f32)
            nc.vector.tensor_tensor(out=dinv[:], in0=rs[:], in1=mask[:], op=OP.mult)
            ndinv = cpool.tile([128, NBCH], f32)
            nc.vector.tensor_scalar(out=ndinv[:], in0=dinv[:], scalar1=-1.0,
                                    scalar2=None, op0=OP.mult)

            # ---- zero tails: utab zero rows + zstage zero blocks ----
            zt0 = cpool.tile([128, 64], f32)
            nc.vector.memset(zt0[:], 0.0)
            nc.sync.dma_start(utab[N:N + 64, :], zt0[:64, 0:HID])
            for b in range(4):
                nc.sync.dma_start(
                    zst[b].rearrange("(t p) c -> p t c", p=128)[:, nblk[b]:, :],
                    zt0[:, None, :],
                )

            # ---- per-layer pieces ----
            xw0 = npool.tile([128, NBCH, HID], f32)
            hbuf = npool.tile([128, NBCH, HID], f32)
            s2buf = npool.tile([128, NBCH, HID], f32)
            zsum = npool.tile([128, NBCH, HID], f32)
            zt_b = [npool.tile([128, nblk[b], HID], f32, tag=f"ztb{b}",
                                name=f"ztb{b}") for b in range(4)]

            def dense_l1():
                for cb in range(NBCH):
                    ps = pspool.tile([128, 32], f32, tag="ps1", space="PSUM")
                    nc.tensor.matmul(ps[:], lhsT=xtt[:, cb * 128:(cb + 1) * 128],
                                     rhs=w1t[:, :], start=True, stop=True)
                    nc.vector.tensor_copy(out=xw0[:, cb, :], in_=ps[:, 0:HID])
                    nc.vector.tensor_tensor(
                        out=hbuf[:, cb, :], in0=ps[:, HID:32],
                        in1=dinv[:, cb:cb + 1].to_broadcast([128, HID]),
                        op=OP.mult)  # hbuf temporarily holds u1

            def send_table(src):
                # src: [128, NBCH, HID] node-major -> uin -> AllGather -> utab
                nc.sync.dma_start(
                    uin.rearrange("(t p) f -> p t f", p=128), src[:])
                nc.gpsimd.collective_compute(
                    "AllGather", OP.bypass, ins=[uin[0:NPC, :]],
                    outs=[utab[0:N, :]], replica_groups=rg)

            def edge_phase():
                goff = 0  # absolute row offset into the global gidx stream
                for b in range(4):
                    nseg = totrows[b] // SEGROWS
                    for s in range(nseg):
                        seg = gpool.tile([128, SEGROWS, 64], f32, tag="gseg")
                        it = ipool.tile([128, SEGROWS * 8], i16, tag="gidx")
                        col0 = (goff + s * SEGROWS) * 8
                        nc.sync.dma_start(it[:], gidx[:, col0:col0 + SEGROWS * 8])
                        for k in range(SEGROWS // CHROWS):
                            nc.gpsimd.dma_gather(
                                seg[:, k * CHROWS:(k + 1) * CHROWS, :],
                                utab_v[:, :],
                                it[:, k * CHROWS * 8:(k + 1) * CHROWS * 8],
                                CHUNK, CHUNK, 64,
                                queue_num=nextq(),
                            )
                        r0s, r1s = s * SEGROWS, (s + 1) * SEGROWS
                        for (r0, dd, nb, zb0) in runs[b]:
                            if r0 < r0s or r0 >= r1s:
                                continue
                            rr = r0 - r0s
                            inap = seg[:, rr:rr + nb * dd, b * HID:(b + 1) * HID]
                            inap = inap.rearrange("p (t jj) f -> p t f jj", jj=dd)
                            nc.vector.tensor_reduce(
                                out=zt_b[b][:, zb0:zb0 + nb, :], in_=inap,
                                axis=AX.X, op=OP.add)
                    goff += totrows[b]
                for b in range(4):
                    nc.sync.dma_start(
                        zst[b].rearrange("(t p) c -> p t c", p=128)[:, 0:nblk[b], 0:HID],
                        zt_b[b][:])

            def combine_phase():
                for k in range(CPADCH):
                    gcs = []
                    for b in range(4):
                        gc = gcpool.tile([128, CHROWS, 64], f32, tag=f"gc{b}")
                        it = ipool.tile([128, 64], i16, tag="cidx")
                        col0 = (b * CLEN + k * CHUNK) // 16
                        nc.sync.dma_start(it[:], cidx[:, col0:col0 + 64])
                        nc.gpsimd.dma_gather(
                            gc[:], zst[b][:, :], it[:], CHUNK, CHUNK, 64,
                            queue_num=nextq())
                        gcs.append(gc)
                    nb_lo = min(CHROWS, NBCH - k * CHROWS)
                    if nb_lo <= 0:
                        continue
                    t01 = gcpool.tile([128, CHROWS, HID], f32, tag="t01")
                    nc.vector.tensor_tensor(out=t01[:], in0=gcs[0][:, :, 0:HID],
                                            in1=gcs[1][:, :, 0:HID], op=OP.add)
                    t23 = gcpool.tile([128, CHROWS, HID], f32, tag="t23")
                    nc.vector.tensor_tensor(out=t23[:], in0=gcs[2][:, :, 0:HID],
                                            in1=gcs[3][:, :, 0:HID], op=OP.add)
                    nc.vector.tensor_tensor(
                        out=zsum[:, k * CHROWS:k * CHROWS + nb_lo, :],
                        in0=t01[:, 0:nb_lo, :], in1=t23[:, 0:nb_lo, :], op=OP.add)

            # ================= layer 1 =================
            dense_l1()
            send_table(hbuf)          # u1
            edge_phase()
            combine_phase()
            # h = relu(xw0 + b1 + ndinv*z)
            tmp = npool.tile([128, NBCH, HID], f32, tag="tmp")
            nc.vector.tensor_tensor(
                out=tmp[:], in0=zsum[:],
                in1=ndinv[:, :, None].to_broadcast([128, NBCH, HID]), op=OP.mult)
            nc.vector.tensor_tensor(out=tmp[:], in0=tmp[:], in1=xw0[:], op=OP.add)
            nc.vector.tensor_tensor(
                out=tmp[:], in0=tmp[:],
                in1=b1tt[:, None, :].to_broadcast([128, NBCH, HID]), op=OP.add)
            nc.scalar.activation(hbuf[:], tmp[:], AF.Relu)

            # ================= layer 2 =================
            u2 = npool.tile([128, NBCH, HID], f32, tag="tmp2")
            nc.vector.tensor_tensor(
                out=u2[:], in0=hbuf[:],
                in1=dinv[:, :, None].to_broadcast([128, NBCH, HID]), op=OP.mult)
            send_table(u2)
            edge_phase()
            combine_phase()
            nc.vector.tensor_tensor(
                out=s2buf[:], in0=zsum[:],
                in1=ndinv[:, :, None].to_broadcast([128, NBCH, HID]), op=OP.mult)

            # ---- transposes: hsT = [h.T ; s2.T] [32, NPAD] ----
            hsT = bigpool.tile([128, NPAD], f32, tag="big")
            nc.vector.memset(hsT[0:32, :], 0.0)
            for cb in range(NBCH):
                pt = ps2pool.tile([HID, 128], f32, tag="pt", space="PSUM")
                nc.tensor.transpose(pt[:], hbuf[:, cb, :], ident[:])
                nc.vector.tensor_copy(out=hsT[0:HID, cb * 128:(cb + 1) * 128],
                                      in_=pt[:])
                pt2 = ps2pool.tile([HID, 128], f32, tag="pt2", space="PSUM")
                nc.tensor.transpose(pt2[:], s2buf[:, cb, :], ident[:])
                nc.vector.tensor_copy(out=hsT[32:48, cb * 128:(cb + 1) * 128],
                                      in_=pt2[:])

            # ---- final matmul + log_softmax ----
            for cb in range(NBCH):
                po = pspool.tile([128, FIN], f32, tag="po", space="PSUM")
                nc.tensor.matmul(po[:], lhsT=hsT[0:48, cb * 128:(cb + 1) * 128],
                                 rhs=w2t[:, :], start=True, stop=True)
                sb = smpool.tile([128, FIN], f32, tag="sb")
                nc.vector.tensor_tensor(out=sb[:], in0=po[:], in1=b2tt[:],
                                        op=OP.add)
                m = smpool.tile([128, 1], f32, tag="m")
                nc.vector.tensor_reduce(out=m[:], in_=sb[:], axis=AX.X, op=OP.max)
                negm = smpool.tile([128, 1], f32, tag="negm")
                nc.vector.tensor_scalar(out=negm[:], in0=m[:], scalar1=-1.0,
                                        scalar2=None, op0=OP.mult)
                et = smpool.tile([128, FIN], f32, tag="et")
                ssum = smpool.tile([128, 1], f32, tag="ssum")
                nc.scalar.activation(et[:], sb[:], AF.Exp, bias=negm[:],
                                     accum_out=ssum[:])
                ls = smpool.tile([128, 1], f32, tag="ls")
                nc.scalar.activation(ls[:], ssum[:], AF.Ln)
                tot = smpool.tile([128, 1], f32, tag="tot")
                nc.vector.tensor_tensor(out=tot[:], in0=m[:], in1=ls[:], op=OP.add)
                res = smpool.tile([128, FIN], f32, tag="res")
                nc.vector.tensor_tensor(
                    out=res[:], in0=sb[:],
                    in1=tot[:, 0:1].to_broadcast([128, FIN]), op=OP.subtract)
                nc.sync.dma_start(
                    outp.rearrange("(t p) f -> p t f", p=128)[:, cb, :],
                    res[:, None, :])

    nc.finalize()
    return nc


def kernel(x, edge_index, W1, b1, W2, b2):
    import ntff_hook  # noqa: F401  (installs the NTFF profile hook shim)
    from concourse import bass_utils

    x = np.asarray(x, dtype=np.float32)
    W1 = np.asarray(W1, dtype=np.float32)
    b1 = np.asarray(b1, dtype=np.float32)
    W2 = np.asarray(W2, dtype=np.float32)
    b2 = np.asarray(b2, dtype=np.float32)

    deg, gidx_all, cidx_all, struct = _host_prep(edge_index)

    key = ("prog", struct["gl"], tuple(struct["totrows"]), tuple(struct["nblk"]))
    if key not in _CACHE:
        _CACHE[key] = _build(struct)
    nc = _CACHE[key]

    NPAD = _ceil(NPC, 128) * 128
    NBCH = NPAD // 128
    w1cat = np.concatenate([W1[0], W1[1]], axis=1).astype(np.float32)  # [128,32]
    w2cat = np.concatenate(
        [W2[0], np.zeros((16, FIN), np.float32), W2[1]], axis=0
    ).astype(np.float32)  # [48,128]: rows 16-31 zero (dead partitions)

    in_maps = []
    for j in range(NC_N):
        xj = x[j * NPC:(j + 1) * NPC]                      # [12500, 128]
        xjp = np.zeros((NPAD, FIN), np.float32)
        xjp[:NPC] = xj
        xt = np.ascontiguousarray(xjp.T)                   # [128, NPAD]
        dj = np.zeros((NPAD,), np.float32)
        dj[:NPC] = deg[j * NPC:(j + 1) * NPC]
        degt = np.ascontiguousarray(dj.reshape(NBCH, 128).T)  # [128, NBCH]
        in_maps.append({
            "xt": xt,
            "degt": degt,
            "w1": w1cat,
            "b1t": np.tile(b1.reshape(1, HID), (128, 1)),
            "w2": w2cat,
            "b2t": np.tile(b2.reshape(1, FIN), (128, 1)),
            "gidx": gidx_all[j],
            "cidx": cidx_all[j],
        })

    import os
    trace = os.environ.get("GNN_TRACE", "0") == "1"
    res = bass_utils.run_bass_kernel_spmd(
        nc, in_maps, core_ids=list(range(NC_N)), trace=trace)
    if trace and res.exec_time_ns:
        print(f"HW exec time: {res.exec_time_ns} ns")
        kernel.last_exec_ns = res.exec_time_ns

    out = np.empty((N, FIN), np.float32)
    for j in range(NC_N):
        out[j * NPC:(j + 1) * NPC] = res.results[j]["outp"][:NPC]
    return out
